# revision 39
# baseline (speedup 1.0000x reference)
"""Trainium2 Bass kernel for causal self-attention (dense transformer block attn).

Reference computation (per batch b):
    qkv = x @ W_attn + b_attn ; split into per-head Q, K, V (16 heads, hs=64)
    att = softmax(mask(Q K^T / sqrt(hs))) ; y = att @ V ; out = y @ W_proj + b_proj

Sharding (8 cores): data parallel on B (2) x tensor parallel on head groups
(4 groups of 4 heads, Megatron-style column/row split of W_attn / W_proj).
Each core computes a partial out^T [1024, 2048] (f32); host sums the 4 partials
per batch, adds b_proj and transposes.

Core kernel layout notes:
  - Everything on-chip is transposed: x^T, qkv^T ([feature, T]), scores are
    computed as S^T = K Q^T with k-positions on partitions so that the PV
    matmul needs no transposes (P^T is the moving operand, V natural the
    stationary).
  - Softmax denominator: the PV stationary is [V | ones] (or [ones | V]) so
    partitions 64..127 (0..63) of the PV psum accumulate 64 copies of
    sum_k P[q,k]; a DMA partition-shift + DVE reciprocal + multiply
    normalizes during psum evacuation.
  - exp() runs on ScalarE straight out of PSUM in wide [128, 2, <=1024]
    instructions (two heads at once) to amortize the ~352-cycle ACT overhead.
"""

import os
import sys

import numpy as np
import ml_dtypes

import concourse.bass as bass
import concourse.tile as tile
import concourse.mybir as mybir
from concourse import bacc
from concourse.bass_utils import run_bass_kernel_spmd

BF16 = mybir.dt.bfloat16
F32 = mybir.dt.float32
AF = mybir.ActivationFunctionType

T = 2048          # sequence length
C = 1024          # model dim
HPC = 4           # heads per core
HS = 64           # head size
NF = 3 * HPC * HS  # per-core qkv features (768)
N_CORES = 8
QB = 512          # q block (psum bank of f32)

bf16 = ml_dtypes.bfloat16


def _chunks512(lo, hi):
    """Split [lo, hi) into pieces that don't cross multiples of 512."""
    out = []
    a = lo
    while a < hi:
        b = min(hi, (a // 512 + 1) * 512)
        out.append((a, b))
        a = b
    return out


def build_kernel():
    nc = bacc.Bacc("TRN2", target_bir_lowering=False, debug=False)

    xT = nc.dram_tensor("xT", [C, T], BF16, kind="ExternalInput").ap()
    W = nc.dram_tensor("W", [C, NF], BF16, kind="ExternalInput").ap()
    bcols = nc.dram_tensor("bcols", [128, 6], F32, kind="ExternalInput").ap()
    Wp = nc.dram_tensor("Wp", [HPC * HS, C], BF16, kind="ExternalInput").ap()
    mask = nc.dram_tensor("mask", [128, 128], BF16, kind="ExternalInput").ap()
    ident = nc.dram_tensor("ident", [128, 128], BF16, kind="ExternalInput").ap()
    outT = nc.dram_tensor("outT", [C, T], F32, kind="ExternalOutput").ap()

    with tile.TileContext(nc) as tc:
        _emit(nc, tc, xT, W, bcols, Wp, mask, ident, outT)
    nc.compile()
    return nc


def _emit(nc, tc, xT, W, bcols, Wp, mask, ident, outT):
    from contextlib import ExitStack

    ctx = ExitStack()
    consts = ctx.enter_context(tc.tile_pool(name="consts", bufs=1))
    pt_pool = ctx.enter_context(tc.tile_pool(name="pt", bufs=1))
    rt_pool = ctx.enter_context(tc.tile_pool(name="rt", bufs=2))
    osb_pool = ctx.enter_context(tc.tile_pool(name="osb", bufs=2))
    ps_s = ctx.enter_context(tc.tile_pool(name="ps_s", bufs=2, space="PSUM"))
    ps_sm = ctx.enter_context(tc.tile_pool(name="ps_sm", bufs=4, space="PSUM"))

    # ---------------- constant / input loads ----------------
    xT_v = xT.rearrange("(c p) t -> p c t", p=128)
    xT_t = consts.tile([128, 8, T], BF16, tag="xT", name="xT_t")
    W_v = W.rearrange("(c p) n -> p c n", p=128)
    W_t = consts.tile([128, 8, NF], BF16, tag="W", name="W_t")
    for c in range(8):
        nc.sync.dma_start(out=W_t[:, c, :], in_=W_v[:, c, :])
        nc.sync.dma_start(out=xT_t[:, c, :], in_=xT_v[:, c, :])
    b_t = consts.tile([128, 6], F32, tag="b", name="b_t")
    nc.sync.dma_start(out=b_t, in_=bcols)
    Wp_t = consts.tile([128, 2, C], BF16, tag="Wp", name="Wp_t")
    nc.sync.dma_start(out=Wp_t, in_=Wp.rearrange("(k p) n -> p k n", p=128))
    mask_t = consts.tile([128, 128], BF16, tag="mask", name="mask_t")
    nc.sync.dma_start(out=mask_t, in_=mask)
    id_t = consts.tile([128, 128], BF16, tag="ident", name="id_t")
    nc.sync.dma_start(out=id_t, in_=ident)

    # DRAM scratch for the reciprocal reshape bounce: [unit, pre/post, 512]
    scr = nc.dram_tensor("pv_scr", [16, 2, QB], F32).ap()

    qkvT = consts.tile([128, 6, T], BF16, tag="qkvT", name="qkvT")
    # vnat[p, pair, j, hl, col]: PV stationary tiles. hl=0: [V | ones],
    # hl=1: [ones | V] so that y lands on the partitions matching yT layout.
    vnat = consts.tile([128, 2, 16, 2, 128], BF16, tag="vnat", name="vnat")
    yT = consts.tile([128, 2, T], BF16, tag="yT", name="yT")

    # warm up the ACT exp table early so the ~2.7us load overlaps the lead-in
    warm = consts.tile([128, 8], F32, tag="warm", name="warm")
    nc.vector.memset(warm, 0.0)
    nc.scalar.activation(warm, warm, AF.Exp, scale=1.0)

    nc.vector.memset(vnat[:, :, :, 0, 64:128], 1.0)
    nc.vector.memset(vnat[:, :, :, 1, 0:64], 1.0)

    # ---------------- phase helpers ----------------
    def qkv_chunk(nf):
        # qkv^T[nf*128:(nf+1)*128, :] = (x @ W[:, cols])^T  (+ bias on evac)
        for qb4 in range(4):
            ps = ps_sm.tile([128, QB], F32, tag="sm", name="ps_qkv")
            for c in range(8):
                nc.tensor.matmul(
                    ps,
                    lhsT=W_t[:, c, nf * 128:(nf + 1) * 128],
                    rhs=xT_t[:, c, qb4 * QB:(qb4 + 1) * QB],
                    start=(c == 0),
                    stop=(c == 7),
                )
            nc.vector.tensor_scalar_add(
                qkvT[:, nf, qb4 * QB:(qb4 + 1) * QB], ps, b_t[:, nf:nf + 1]
            )

    def vtrans(p):
        # V^T tile (qkvT[:, 4+p]) -> natural V chunks in vnat[:, p]
        for j in range(16):
            pst = ps_sm.tile([128, 128], BF16, tag="sm", name="ps_vt")
            nc.tensor.transpose(pst, qkvT[:, 4 + p, j * 128:(j + 1) * 128], id_t)
            # single strided copy: psum cols [0:64|64:128] -> vnat
            # [j, 0, 0:64] and [j, 1, 64:128]
            v0 = vnat[:, p, j, 0, 0:64]
            dst = bass.AP(tensor=v0.tensor, offset=v0.offset,
                          ap=[v0.ap[0], [192, 2], [1, 64]])
            s0 = pst[:, 0:64]
            src = bass.AP(tensor=s0.tensor, offset=s0.offset,
                          ap=[s0.ap[0], [64, 2], [1, 64]])
            nc.vector.tensor_copy(dst, src)

    pt_tiles = {}

    def s_exp(p, j):
        # scores^T for pair p, key chunk j (both heads), then exp -> PT
        wj = T - 128 * j
        pt = pt_pool.tile([128, 2, wj], BF16, tag=f"pt{j}",
                          name=f"pt_{p}_{j}", bufs=2 if j < 2 else 1)
        pt_tiles[(p, j)] = pt
        for qh in range(4):
            qlo = max(128 * j, 512 * qh)
            qhi = 512 * (qh + 1)
            if qlo >= qhi:
                continue
            lo = qlo - 512 * qh
            ps = ps_s.tile([128, 2, 512], F32, tag="s", name="ps_s_t")
            for hl in range(2):
                nc.tensor.matmul(
                    ps[:, hl, lo:(qhi - 512 * qh)],
                    lhsT=qkvT[64 * hl:64 * hl + 64, 2 + p, j * 128:(j + 1) * 128],
                    rhs=qkvT[64 * hl:64 * hl + 64, p, qlo:qhi],
                    start=True,
                    stop=True,
                )
            nc.scalar.activation(
                pt[:, :, (qlo - 128 * j):(qhi - 128 * j)],
                ps[:, :, lo:(qhi - 512 * qh)],
                AF.Exp,
                scale=0.125,
            )
        # zero the q < k upper triangle of the diagonal chunk (both heads in
        # one mul via a broadcast AP over the head dim; GpSimd is idle and
        # this keeps DVE off the exp -> PV critical path)
        mb = bass.AP(tensor=mask_t.tensor, offset=mask_t.offset,
                     ap=[mask_t.ap[0], [0, 2], [1, 128]])
        nc.gpsimd.tensor_mul(pt[:, :, 0:128], pt[:, :, 0:128], mb)

    def pv_unit(p, hl, qb4):
        # y^T (and denominator copies) for head (p, hl), q block qb4.
        ysl = slice(64 * hl, 64 * hl + 64)
        dsl = slice(64 - 64 * hl, 128 - 64 * hl)
        ps = ps_sm.tile([128, QB], F32, tag="sm", name=f"ps_pv{p}{hl}")
        last = 4 * qb4 + 3
        for jp in range(0, last + 1):
            pt = pt_tiles[(p, jp)]
            qlo = max(qb4 * QB, 128 * jp)
            qhi = qb4 * QB + QB
            nc.tensor.matmul(
                ps[:, (qlo - qb4 * QB):(qhi - qb4 * QB)],
                lhsT=vnat[:, p, jp, hl, :],
                rhs=pt[:, hl, (qlo - 128 * jp):(qhi - 128 * jp)],
                start=(jp == 0),
                stop=(jp == last),
            )
        # One fast copy frees the psum bank. InstReciprocal cost scales with
        # FREE size only (~6.5 cyc/elem), so bounce one denominator row
        # through DRAM to reshape [1,512] -> [128,4], recip there (~30ns),
        # and bounce back with a partition-broadcast to the y rows.
        uid = (p * 2 + hl) * 4 + qb4
        sb = rt_pool.tile([128, QB], F32, tag="sb", name="sb")
        nc.vector.tensor_copy(sb, ps)
        # bounce DMAs issue from the otherwise-idle GpSimd queue so they
        # don't serialize behind input/output streaming on the Sync queue
        nc.gpsimd.dma_start(out=scr[uid, 0, :], in_=sb[dsl.start:dsl.start + 1, :])
        rtp = rt_pool.tile([128, 4], F32, tag="rtp", name="rtp")
        nc.gpsimd.dma_start(out=rtp, in_=scr[uid, 0, :].rearrange("(a f) -> a f", f=4))
        rtq = rt_pool.tile([128, 4], F32, tag="rtq", name="rtq")
        nc.vector.reciprocal(rtq, rtp)
        nc.gpsimd.dma_start(out=scr[uid, 1, :].rearrange("(a f) -> a f", f=4), in_=rtq)
        s1 = scr[uid, 1, :]
        bcast = bass.AP(tensor=s1.tensor, offset=s1.offset, ap=[[0, 64], [1, QB]])
        rt2 = rt_pool.tile([128, QB], F32, tag="rt2", name="rt2")
        nc.gpsimd.dma_start(out=rt2[ysl, :], in_=bcast)
        nc.vector.tensor_mul(
            yT[ysl, p, qb4 * QB:(qb4 + 1) * QB], sb[ysl, :], rt2[ysl, :]
        )

    outT_v = outT.rearrange("(n p) t -> p n t", p=128)

    def proj_qb(qb4):
        # final projection for one q block (needs yT of both pairs for it)
        qsl = slice(qb4 * QB, (qb4 + 1) * QB)
        for nf2 in range(4):
            ob = osb_pool.tile([128, 2, QB], F32, tag="osb", name="ob")
            for sub in range(2):
                nf = nf2 * 2 + sub
                ps = ps_sm.tile([128, QB], F32, tag="sm", name="ps_o")
                for kc in range(2):
                    nc.tensor.matmul(
                        ps,
                        lhsT=Wp_t[:, kc, nf * 128:(nf + 1) * 128],
                        rhs=yT[:, kc, qsl],
                        start=(kc == 0),
                        stop=(kc == 1),
                    )
                nc.scalar.copy(ob[:, sub, :], ps)
            nc.sync.dma_start(out=outT_v[:, nf2 * 2:nf2 * 2 + 2, qsl], in_=ob)

    # ---------------- emission schedule ----------------
    # lead: qkv chunks needed by pair 0 (Q01, K01, V01), its V transpose
    with nc.named_scope("lead"):
        for nf in (0, 2, 4):
            qkv_chunk(nf)
        vtrans(0)
    # pair-0 scores/exp interleaved with pair-1 qkv and pair-0 PV units.
    # PV/proj blocks are lagged a step or two behind the exp that feeds them
    # so the in-order PE queue never blocks on a fresh ACT/DVE result.
    with nc.named_scope("pair0"):
        for j in range(16):
            s_exp(0, j)
            if j == 0:
                qkv_chunk(1)
            if j == 1:
                qkv_chunk(3)
            if j >= 4 and j % 4 == 0:
                qb4 = j // 4 - 1
                pv_unit(0, 0, qb4)
                pv_unit(0, 1, qb4)
    # pair-1 scores/exp interleaved with pair-0's last PV unit, its own PV
    # units, and the final projection (per q block, as soon as both pairs'
    # yT for that block is done)
    with nc.named_scope("pair1"):
        for j in range(16):
            if j == 0:
                pv_unit(0, 0, 3)
                pv_unit(0, 1, 3)
            s_exp(1, j)
            if j == 0:
                qkv_chunk(5)
            if j == 1:
                vtrans(1)
            if j >= 4 and j % 4 == 0:
                qb4 = j // 4 - 1
                pv_unit(1, 0, qb4)
                pv_unit(1, 1, qb4)
            if j >= 7 and j % 4 == 3:
                with nc.named_scope("proj"):
                    proj_qb(j // 4 - 1)
    with nc.named_scope("tail"):
        pv_unit(1, 0, 3)
        pv_unit(1, 1, 3)
        proj_qb(3)
    ctx.close()


# ---------------------------------------------------------------------------
# host-side wrapper
# ---------------------------------------------------------------------------

_NC_CACHE = {}


def _get_nc():
    if "nc" not in _NC_CACHE:
        _NC_CACHE["nc"] = build_kernel()
    return _NC_CACHE["nc"]


def make_in_maps(x, W_attn, b_attn, W_proj, b_proj):
    B = x.shape[0]
    # multiplicative causal mask for the diagonal chunk, [k, q]: 1 where q >= k
    mask_np = np.triu(np.ones((128, 128), np.float32)).astype(bf16)
    ident_np = np.eye(128, dtype=np.float32).astype(bf16)
    in_maps = []
    for core in range(N_CORES):
        b = core // 4
        g = core % 4
        cols = np.r_[256 * g:256 * g + 256,
                     1024 + 256 * g:1024 + 256 * g + 256,
                     2048 + 256 * g:2048 + 256 * g + 256]
        in_maps.append({
            "xT": np.ascontiguousarray(x[b].T).astype(bf16),
            "W": np.ascontiguousarray(W_attn[:, cols]).astype(bf16),
            "bcols": np.ascontiguousarray(
                b_attn[cols].reshape(6, 128).T).astype(np.float32),
            "Wp": np.ascontiguousarray(
                W_proj[256 * g:256 * g + 256, :]).astype(bf16),
            "mask": mask_np,
            "ident": ident_np,
        })
    return in_maps


def kernel(x, W_attn, b_attn, W_proj, b_proj, _trace=False, _trace_kwargs=None):
    x = np.asarray(x, np.float32)
    W_attn = np.asarray(W_attn, np.float32)
    b_attn = np.asarray(b_attn, np.float32)
    W_proj = np.asarray(W_proj, np.float32)
    b_proj = np.asarray(b_proj, np.float32)

    nc = _get_nc()
    in_maps = make_in_maps(x, W_attn, b_attn, W_proj, b_proj)
    res = run_bass_kernel_spmd(
        nc, in_maps, core_ids=list(range(N_CORES)), trace=_trace,
        **(_trace_kwargs or {}),
    )
    B = x.shape[0]
    out = np.zeros((B, T, C), np.float32)
    for core in range(N_CORES):
        b = core // 4
        out[b] += res.results[core]["outT"].T
    out += b_proj[None, None, :]
    if _trace:
        kernel._last_results = res
    return out


if __name__ == "__main__":
    # smoke test: build only
    nc = build_kernel()
    print("built ok")


# revision 45
# speedup vs baseline: 1.1550x; 1.1550x over previous
"""Trainium2 Bass kernel for causal self-attention (dense transformer block attn).

Reference computation (per batch b):
    qkv = x @ W_attn + b_attn ; split into per-head Q, K, V (16 heads, hs=64)
    att = softmax(mask(Q K^T / sqrt(hs))) ; y = att @ V ; out = y @ W_proj + b_proj

Sharding (8 cores): data parallel on B (2) x tensor parallel on head groups
(4 groups of 4 heads, Megatron-style column/row split of W_attn / W_proj).
Each core computes a partial out^T [1024, 2048] (f32); host sums the 4 partials
per batch, adds b_proj and transposes.

Core kernel layout notes:
  - Everything on-chip is transposed: x^T, qkv^T ([feature, T]), scores are
    computed as S^T = K Q^T with k-positions on partitions so that the PV
    matmul needs no transposes (P^T is the moving operand, V natural the
    stationary).
  - Softmax denominator: the PV stationary is [V | ones] (or [ones | V]) so
    partitions 64..127 (0..63) of the PV psum accumulate 64 copies of
    sum_k P[q,k]; a DMA partition-shift + DVE reciprocal + multiply
    normalizes during psum evacuation.
  - exp() runs on ScalarE straight out of PSUM in wide [128, 2, <=1024]
    instructions (two heads at once) to amortize the ~352-cycle ACT overhead.
"""

import os
import sys

import numpy as np
import ml_dtypes

import concourse.bass as bass
import concourse.tile as tile
import concourse.mybir as mybir
from concourse import bacc
from concourse.bass_utils import run_bass_kernel_spmd

BF16 = mybir.dt.bfloat16
F32 = mybir.dt.float32
AF = mybir.ActivationFunctionType

T = 2048          # sequence length
C = 1024          # model dim
HPC = 4           # heads per core
HS = 64           # head size
NF = 3 * HPC * HS  # per-core qkv features (768)
N_CORES = 8
QB = 512          # q block (psum bank of f32)

bf16 = ml_dtypes.bfloat16


def _chunks512(lo, hi):
    """Split [lo, hi) into pieces that don't cross multiples of 512."""
    out = []
    a = lo
    while a < hi:
        b = min(hi, (a // 512 + 1) * 512)
        out.append((a, b))
        a = b
    return out


def build_kernel():
    nc = bacc.Bacc("TRN2", target_bir_lowering=False, debug=False)

    xT = nc.dram_tensor("xT", [C, T], BF16, kind="ExternalInput").ap()
    W = nc.dram_tensor("W", [C, NF], BF16, kind="ExternalInput").ap()
    bcols = nc.dram_tensor("bcols", [128, 6], F32, kind="ExternalInput").ap()
    Wp = nc.dram_tensor("Wp", [HPC * HS, C], BF16, kind="ExternalInput").ap()
    mask = nc.dram_tensor("mask", [128, 128], BF16, kind="ExternalInput").ap()
    ident = nc.dram_tensor("ident", [128, 128], BF16, kind="ExternalInput").ap()
    outT = nc.dram_tensor("outT", [C, T], F32, kind="ExternalOutput").ap()

    with tile.TileContext(nc) as tc:
        _emit(nc, tc, xT, W, bcols, Wp, mask, ident, outT)
    nc.compile()
    return nc


def _emit(nc, tc, xT, W, bcols, Wp, mask, ident, outT):
    from contextlib import ExitStack

    ctx = ExitStack()
    consts = ctx.enter_context(tc.tile_pool(name="consts", bufs=1))
    pt_pool = ctx.enter_context(tc.tile_pool(name="pt", bufs=1))
    rt_pool = ctx.enter_context(tc.tile_pool(name="rt", bufs=2))
    osb_pool = ctx.enter_context(tc.tile_pool(name="osb", bufs=2))
    ps_s = ctx.enter_context(tc.tile_pool(name="ps_s", bufs=2, space="PSUM"))
    ps_sm = ctx.enter_context(tc.tile_pool(name="ps_sm", bufs=3, space="PSUM"))
    ps_junk = ctx.enter_context(tc.tile_pool(name="ps_junk", bufs=1, space="PSUM"))

    # ---------------- constant / input loads ----------------
    xT_v = xT.rearrange("(c p) t -> p c t", p=128)
    xT_t = consts.tile([128, 8, T], BF16, tag="xT", name="xT_t")
    W_v = W.rearrange("(c p) n -> p c n", p=128)
    W_t = consts.tile([128, 8, NF], BF16, tag="W", name="W_t")
    for c in range(8):
        nc.sync.dma_start(out=W_t[:, c, :], in_=W_v[:, c, :])
        nc.sync.dma_start(out=xT_t[:, c, :], in_=xT_v[:, c, :])
    b_t = consts.tile([128, 6], F32, tag="b", name="b_t")
    nc.sync.dma_start(out=b_t, in_=bcols)
    Wp_t = consts.tile([128, 2, C], BF16, tag="Wp", name="Wp_t")
    nc.sync.dma_start(out=Wp_t, in_=Wp.rearrange("(k p) n -> p k n", p=128))
    mask_t = consts.tile([128, 128], BF16, tag="mask", name="mask_t")
    nc.sync.dma_start(out=mask_t, in_=mask)
    id_t = consts.tile([128, 128], BF16, tag="ident", name="id_t")
    nc.sync.dma_start(out=id_t, in_=ident)

    # DRAM scratch for the reciprocal reshape bounce: [unit, pre/post, 512]
    scr = nc.dram_tensor("pv_scr", [16, 2, QB], F32).ap()

    qkvT = consts.tile([128, 6, T], BF16, tag="qkvT", name="qkvT")
    # vnat[p, pair, j, hl, col]: PV stationary tiles. hl=0: [V | ones],
    # hl=1: [ones | V] so that y lands on the partitions matching yT layout.
    vnat = consts.tile([128, 2, 16, 2, 128], BF16, tag="vnat", name="vnat")
    yT = consts.tile([128, 2, T], BF16, tag="yT", name="yT")

    # warm up the ACT exp table early so the ~2.7us load overlaps the lead-in
    warm = consts.tile([128, 8], F32, tag="warm", name="warm")
    nc.vector.memset(warm, 0.0)
    nc.scalar.activation(warm, warm, AF.Exp, scale=1.0)

    # dependency-free junk matmuls: keep the PE activity monitor from
    # re-throttling the clock (K=4/8) across short cross-engine stalls
    junk = ps_junk.tile([128, QB], F32, tag="junk", name="junk")

    def keep_warm(n=2):
        for _ in range(n):
            nc.tensor.matmul(junk, lhsT=W_t[:, 0, 0:128],
                             rhs=xT_t[:, 0, 0:QB], start=True, stop=True)

    nc.vector.memset(vnat[:, :, :, 0, 64:128], 1.0)
    nc.vector.memset(vnat[:, :, :, 1, 0:64], 1.0)

    # ---------------- phase helpers ----------------
    def qkv_chunk(nf):
        # qkv^T[nf*128:(nf+1)*128, :] = (x @ W[:, cols])^T  (+ bias on evac)
        for qb4 in range(4):
            ps = ps_sm.tile([128, QB], F32, tag="sm", name="ps_qkv")
            for c in range(8):
                nc.tensor.matmul(
                    ps,
                    lhsT=W_t[:, c, nf * 128:(nf + 1) * 128],
                    rhs=xT_t[:, c, qb4 * QB:(qb4 + 1) * QB],
                    start=(c == 0),
                    stop=(c == 7),
                )
            nc.vector.tensor_scalar_add(
                qkvT[:, nf, qb4 * QB:(qb4 + 1) * QB], ps, b_t[:, nf:nf + 1]
            )

    def vtrans(p):
        # V^T tile (qkvT[:, 4+p]) -> natural V chunks in vnat[:, p]
        for j in range(16):
            pst = ps_sm.tile([128, 128], BF16, tag="sm", name="ps_vt")
            nc.tensor.transpose(pst, qkvT[:, 4 + p, j * 128:(j + 1) * 128], id_t)
            # single strided copy: psum cols [0:64|64:128] -> vnat
            # [j, 0, 0:64] and [j, 1, 64:128]
            v0 = vnat[:, p, j, 0, 0:64]
            dst = bass.AP(tensor=v0.tensor, offset=v0.offset,
                          ap=[v0.ap[0], [192, 2], [1, 64]])
            s0 = pst[:, 0:64]
            src = bass.AP(tensor=s0.tensor, offset=s0.offset,
                          ap=[s0.ap[0], [64, 2], [1, 64]])
            nc.vector.tensor_copy(dst, src)

    pt_tiles = {}

    def s_exp(p, j):
        # scores^T for pair p, key chunk j (both heads), then exp -> PT
        wj = T - 128 * j
        pt = pt_pool.tile([128, 2, wj], BF16, tag=f"pt{j}",
                          name=f"pt_{p}_{j}", bufs=2 if j < 2 else 1)
        pt_tiles[(p, j)] = pt
        for qh in range(4):
            qlo = max(128 * j, 512 * qh)
            qhi = 512 * (qh + 1)
            if qlo >= qhi:
                continue
            lo = qlo - 512 * qh
            ps = ps_s.tile([128, 2, 512], F32, tag="s", name="ps_s_t")
            for hl in range(2):
                nc.tensor.matmul(
                    ps[:, hl, lo:(qhi - 512 * qh)],
                    lhsT=qkvT[64 * hl:64 * hl + 64, 2 + p, j * 128:(j + 1) * 128],
                    rhs=qkvT[64 * hl:64 * hl + 64, p, qlo:qhi],
                    start=True,
                    stop=True,
                )
            nc.scalar.activation(
                pt[:, :, (qlo - 128 * j):(qhi - 128 * j)],
                ps[:, :, lo:(qhi - 512 * qh)],
                AF.Exp,
                scale=0.125,
            )
        # zero the q < k upper triangle of the diagonal chunk (both heads in
        # one mul via a broadcast AP over the head dim; GpSimd is idle and
        # this keeps DVE off the exp -> PV critical path)
        mb = bass.AP(tensor=mask_t.tensor, offset=mask_t.offset,
                     ap=[mask_t.ap[0], [0, 2], [1, 128]])
        nc.vector.tensor_mul(pt[:, :, 0:128], pt[:, :, 0:128], mb)

    def pv_unit(p, hl, qb4):
        # y^T (and denominator copies) for head (p, hl), q block qb4.
        ysl = slice(64 * hl, 64 * hl + 64)
        dsl = slice(64 - 64 * hl, 128 - 64 * hl)
        ps = ps_sm.tile([128, QB], F32, tag="sm", name=f"ps_pv{p}{hl}")
        last = 4 * qb4 + 3
        for jp in range(0, last + 1):
            pt = pt_tiles[(p, jp)]
            qlo = max(qb4 * QB, 128 * jp)
            qhi = qb4 * QB + QB
            nc.tensor.matmul(
                ps[:, (qlo - qb4 * QB):(qhi - qb4 * QB)],
                lhsT=vnat[:, p, jp, hl, :],
                rhs=pt[:, hl, (qlo - 128 * jp):(qhi - 128 * jp)],
                start=(jp == 0),
                stop=(jp == last),
            )
        # One fast copy frees the psum bank. InstReciprocal cost scales with
        # FREE size only (~6.5 cyc/elem), so bounce one denominator row
        # through DRAM to reshape [1,512] -> [128,4], recip there (~30ns),
        # and bounce back with a partition-broadcast to the y rows.
        uid = (p * 2 + hl) * 4 + qb4
        sb = rt_pool.tile([128, QB], F32, tag="sb", name="sb")
        nc.vector.tensor_copy(sb, ps)
        # bounce DMAs issue from the otherwise-idle GpSimd queue so they
        # don't serialize behind input/output streaming on the Sync queue
        nc.sync.dma_start(out=scr[uid, 0, :], in_=sb[dsl.start:dsl.start + 1, :])
        rtp = rt_pool.tile([128, 4], F32, tag="rtp", name="rtp")
        nc.sync.dma_start(out=rtp, in_=scr[uid, 0, :].rearrange("(a f) -> a f", f=4))
        rtq = rt_pool.tile([128, 4], F32, tag="rtq", name="rtq")
        nc.vector.reciprocal(rtq, rtp)
        nc.sync.dma_start(out=scr[uid, 1, :].rearrange("(a f) -> a f", f=4), in_=rtq)
        s1 = scr[uid, 1, :]
        bcast = bass.AP(tensor=s1.tensor, offset=s1.offset, ap=[[0, 64], [1, QB]])
        rt2 = rt_pool.tile([128, QB], F32, tag="rt2", name="rt2")
        nc.sync.dma_start(out=rt2[ysl, :], in_=bcast)
        nc.vector.tensor_mul(
            yT[ysl, p, qb4 * QB:(qb4 + 1) * QB], sb[ysl, :], rt2[ysl, :]
        )

    outT_v = outT.rearrange("(n p) t -> p n t", p=128)

    def proj_qb(qb4):
        # final projection for one q block (needs yT of both pairs for it)
        qsl = slice(qb4 * QB, (qb4 + 1) * QB)
        for nf2 in range(4):
            ob = osb_pool.tile([128, 2, QB], F32, tag="osb", name="ob")
            for sub in range(2):
                nf = nf2 * 2 + sub
                ps = ps_sm.tile([128, QB], F32, tag="sm", name="ps_o")
                for kc in range(2):
                    nc.tensor.matmul(
                        ps,
                        lhsT=Wp_t[:, kc, nf * 128:(nf + 1) * 128],
                        rhs=yT[:, kc, qsl],
                        start=(kc == 0),
                        stop=(kc == 1),
                    )
                nc.scalar.copy(ob[:, sub, :], ps)
            nc.sync.dma_start(out=outT_v[:, nf2 * 2:nf2 * 2 + 2, qsl], in_=ob)

    # ---------------- emission schedule ----------------
    # lead: qkv chunks needed by pair 0 (Q01, K01, V01), its V transpose
    with nc.named_scope("lead"):
        for nf in (0, 2, 4):
            qkv_chunk(nf)
        vtrans(0)
    # pair-0 scores/exp interleaved with pair-1 qkv and pair-0 PV units.
    # PV/proj blocks are lagged a step or two behind the exp that feeds them
    # so the in-order PE queue never blocks on a fresh ACT/DVE result.
    with nc.named_scope("pair0"):
        for j in range(16):
            s_exp(0, j)
            if j == 0:
                qkv_chunk(1)
            if j == 1:
                qkv_chunk(3)
            if j >= 4 and j % 4 == 0:
                qb4 = j // 4 - 1
                keep_warm(2)
                pv_unit(0, 0, qb4)
                pv_unit(0, 1, qb4)
            if j >= 8:
                keep_warm(2)
    # pair-1 scores/exp interleaved with pair-0's last PV unit, its own PV
    # units, and the final projection (per q block, as soon as both pairs'
    # yT for that block is done)
    with nc.named_scope("pair1"):
        for j in range(16):
            if j == 0:
                pv_unit(0, 0, 3)
                pv_unit(0, 1, 3)
            s_exp(1, j)
            if j == 0:
                qkv_chunk(5)
            if j == 1:
                vtrans(1)
            if j >= 4 and j % 4 == 0:
                qb4 = j // 4 - 1
                keep_warm(2)
                pv_unit(1, 0, qb4)
                pv_unit(1, 1, qb4)
            if j >= 7 and j % 4 == 3:
                keep_warm(2)
                with nc.named_scope("proj"):
                    proj_qb(j // 4 - 1)
            if j >= 8:
                keep_warm(2)
    with nc.named_scope("tail"):
        keep_warm(2)
        pv_unit(1, 0, 3)
        pv_unit(1, 1, 3)
        keep_warm(4)
        proj_qb(3)
    ctx.close()


# ---------------------------------------------------------------------------
# host-side wrapper
# ---------------------------------------------------------------------------

_NC_CACHE = {}


def _get_nc():
    if "nc" not in _NC_CACHE:
        _NC_CACHE["nc"] = build_kernel()
    return _NC_CACHE["nc"]


def make_in_maps(x, W_attn, b_attn, W_proj, b_proj):
    B = x.shape[0]
    # multiplicative causal mask for the diagonal chunk, [k, q]: 1 where q >= k
    mask_np = np.triu(np.ones((128, 128), np.float32)).astype(bf16)
    ident_np = np.eye(128, dtype=np.float32).astype(bf16)
    in_maps = []
    for core in range(N_CORES):
        b = core // 4
        g = core % 4
        cols = np.r_[256 * g:256 * g + 256,
                     1024 + 256 * g:1024 + 256 * g + 256,
                     2048 + 256 * g:2048 + 256 * g + 256]
        in_maps.append({
            "xT": np.ascontiguousarray(x[b].T).astype(bf16),
            "W": np.ascontiguousarray(W_attn[:, cols]).astype(bf16),
            "bcols": np.ascontiguousarray(
                b_attn[cols].reshape(6, 128).T).astype(np.float32),
            "Wp": np.ascontiguousarray(
                W_proj[256 * g:256 * g + 256, :]).astype(bf16),
            "mask": mask_np,
            "ident": ident_np,
        })
    return in_maps


def kernel(x, W_attn, b_attn, W_proj, b_proj, _trace=False, _trace_kwargs=None):
    x = np.asarray(x, np.float32)
    W_attn = np.asarray(W_attn, np.float32)
    b_attn = np.asarray(b_attn, np.float32)
    W_proj = np.asarray(W_proj, np.float32)
    b_proj = np.asarray(b_proj, np.float32)

    nc = _get_nc()
    in_maps = make_in_maps(x, W_attn, b_attn, W_proj, b_proj)
    res = run_bass_kernel_spmd(
        nc, in_maps, core_ids=list(range(N_CORES)), trace=_trace,
        **(_trace_kwargs or {}),
    )
    B = x.shape[0]
    out = np.zeros((B, T, C), np.float32)
    for core in range(N_CORES):
        b = core // 4
        out[b] += res.results[core]["outT"].T
    out += b_proj[None, None, :]
    if _trace:
        kernel._last_results = res
    return out


if __name__ == "__main__":
    # smoke test: build only
    nc = build_kernel()
    print("built ok")
